# revision 1
# baseline (speedup 1.0000x reference)
"""Bass/Tile SPMD kernel for GPT2 non-residual attention (v2).

Sharding: core c -> (batch b=c//2, half=c%2). Each core computes 4 q-blocks
(128 rows each) of its batch: half0 -> blocks [0,3,4,7], half1 -> [1,2,5,6].
Uniform slot key-extents [2,4,6,8] key-blocks. Causal/pad masking is a
multiplicative {0,1} bf16 mask applied to exp(scores) for the last two
key-blocks of each slot (per-core input data keeps the program SPMD-uniform).

All matmuls bf16 with f32 PSUM accumulation. The computed k,v feed only the
diagonal self term, so row-sharding needs no collectives. The softmax
denominator comes free from a ones-column appended to V.

PSUM discipline: PE row-tiles T0 (SBUF partitions 0-63) and T8 (64-127) run
concurrently and must never write the same PSUM bank; every bank here is
written by exactly one parity.
"""
import numpy as np
import ml_dtypes

import concourse.bacc as bacc
import concourse.mybir as mybir
import concourse.tile as tile
from concourse.bass_utils import run_bass_kernel_spmd
from concourse.masks import make_identity

BF = mybir.dt.bfloat16
F32 = mybir.dt.float32
AF = mybir.ActivationFunctionType

B, S, E, H, DH, P = 4, 1024, 1024, 16, 64, 64
NC = 8
QBLOCKS = [[0, 3, 4, 7], [1, 2, 5, 6]]
EXT = [2, 4, 6, 8]          # key-block extent per slot (uniform across cores)
SCALE = 0.125               # 1/sqrt(DH)

bf16 = ml_dtypes.bfloat16


def build_program(has_bias=False, has_pmask=False, bench_iters=0):
    nc = bacc.Bacc("TRN2", target_bir_lowering=False, debug=False)

    d_xT = nc.dram_tensor("xT", [128, 8, 512], BF, kind="ExternalInput").ap()
    d_W = nc.dram_tensor("W", [128, 8, 3072], BF, kind="ExternalInput").ap()
    d_Wp = nc.dram_tensor("Wp", [128, 8, 1024], BF, kind="ExternalInput").ap()
    d_KT = nc.dram_tensor("KT", [128, 8, 1024], BF, kind="ExternalInput").ap()
    d_Vp = nc.dram_tensor("Vp", [128, 16, 8, 65], BF, kind="ExternalInput").ap()
    d_pKT = nc.dram_tensor("pKT", [128, 8, 64], BF, kind="ExternalInput").ap()
    d_pVp = nc.dram_tensor("pVp", [64, 16, 65], BF, kind="ExternalInput").ap()
    d_mQ = nc.dram_tensor("mQ", [128, 4, 256], BF, kind="ExternalInput").ap()
    if has_bias:
        d_Wb = nc.dram_tensor("Wb", [1, 3072], BF, kind="ExternalInput").ap()
        d_Wpb = nc.dram_tensor("Wpb", [1, 1024], BF, kind="ExternalInput").ap()
    if has_pmask:
        d_pM = nc.dram_tensor("pM", [64, 4, 128], F32, kind="ExternalInput").ap()
    d_out = nc.dram_tensor("out", [4, 128, 1024], F32, kind="ExternalOutput").ap()

    from contextlib import ExitStack
    with tile.TileContext(nc) as tc, ExitStack() as stack:
        res = stack.enter_context(tc.tile_pool(name="res", bufs=1))
        ps = stack.enter_context(tc.tile_pool(name="ps", bufs=8, space="PSUM"))
        if bench_iters:
            loop = stack.enter_context(tc.For_i(0, bench_iters, 1))

        # ---- resident tensors ----
        # W chunked per k-tile so slot-0 matmuls start as soon as chunk 0
        # lands; big loads spread across engines for parallel DMA queues.
        # W/xT live in a scoped pool released after the projection phase
        # (kept resident in bench mode, where the body re-runs in a loop).
        wpool = res if bench_iters else tc.alloc_tile_pool(name="wpool", bufs=1)
        W_s = wpool.tile([128, 8 * 3072], BF, tag="W")
        for kt in range(8):
            for ch in range(2):
                nc.sync.dma_start(
                    W_s[:, kt * 3072 + 1536 * ch:kt * 3072 + 1536 * ch + 1536],
                    d_W[:].rearrange("p k c -> p k c")[:, kt,
                                                      1536 * ch:1536 * ch + 1536])
        xT_s = wpool.tile([128, 8 * 512], BF, tag="xT")
        nc.gpsimd.dma_start(xT_s[:].rearrange("p (k r) -> p k r", k=8), d_xT[:])
        Wp_s = res.tile([128, 8 * 1024], BF, tag="Wp")
        nc.sync.dma_start(Wp_s[:].rearrange("p (k c) -> p k c", k=8), d_Wp[:])
        KT_s = res.tile([128, 8 * 1024], BF, tag="KT")
        nc.gpsimd.dma_start(KT_s[:].rearrange("p (a k) -> p a k", a=8), d_KT[:])
        Vp_s = res.tile([128, 16 * 8 * 65], BF, tag="Vp")
        nc.gpsimd.dma_start(
            Vp_s[:].rearrange("p (h k c) -> p h k c", h=16, k=8), d_Vp[:])
        pKT_s = res.tile([128, 8 * 64], BF, tag="pKT")
        nc.gpsimd.dma_start(pKT_s[:].rearrange("p (a k) -> p a k", a=8), d_pKT[:])
        pVp_s = res.tile([64, 16 * 65], BF, tag="pVp")
        nc.gpsimd.dma_start(pVp_s[:].rearrange("p (h c) -> p h c", h=16), d_pVp[:])
        mQ_s = res.tile([128, 4 * 256], BF, tag="mQ")
        nc.gpsimd.dma_start(mQ_s[:].rearrange("p (s r) -> p s r", s=4), d_mQ[:])
        if has_bias:
            Wb_s = res.tile([1, 3072], BF, tag="Wb")
            nc.sync.dma_start(Wb_s[:], d_Wb[:])
            Wpb_s = res.tile([1, 1024], BF, tag="Wpb")
            nc.sync.dma_start(Wpb_s[:], d_Wpb[:])
        if has_pmask:
            pM_s = res.tile([64, 4 * 128], F32, tag="pM")
            nc.sync.dma_start(pM_s[:].rearrange("p (s r) -> p s r", s=4), d_pM[:])

        ident = res.tile([128, 128], BF, tag="ident")
        make_identity(nc, ident[:])
        ones_col = res.tile([128, 1], BF, tag="ones_col")
        nc.vector.memset(ones_col[:], 1.0)
        ones_row = res.tile([1, 512], BF, tag="ones_row")
        nc.vector.memset(ones_row[:], 1.0)

        # ---------- A. projection for ALL slots upfront ----------
        # qkT_all col-tile t (q: t<8 = pairs, k: t>=8): [128, 512 rows(4 slots)]
        qkT_all = res.tile([128, 16 * 512], BF, tag="qkTa")
        v_all = res.tile([128, 4 * 1024], BF, tag="va")
        for t in range(16):
            pq = ps.tile([128, 512], F32, tag="ps")
            for kt in range(8):
                nc.tensor.matmul(
                    pq[:], W_s[:, kt * 3072 + 128 * t: kt * 3072 + 128 * t + 128],
                    xT_s[:, kt * 512:kt * 512 + 512],
                    start=(kt == 0), stop=(kt == 7 and not has_bias))
            if has_bias:
                nc.tensor.matmul(
                    pq[:], Wb_s[:, 128 * t:128 * t + 128], ones_row[:],
                    start=False, stop=True)
            nc.vector.tensor_copy(qkT_all[:, 512 * t:512 * t + 512], pq[:])
        for s4 in range(4):
            for g in range(2):
                pv = ps.tile([128, 512], F32, tag="ps")
                for kt in range(8):
                    nc.tensor.matmul(
                        pv[:], xT_s[:, kt * 512 + 128 * s4: kt * 512 + 128 * s4 + 128],
                        W_s[:, kt * 3072 + 2048 + 512 * g: kt * 3072 + 2048 + 512 * g + 512],
                        start=(kt == 0), stop=(kt == 7 and not has_bias))
                if has_bias:
                    nc.tensor.matmul(
                        pv[:], ones_row[0:1, 128 * s4:128 * s4 + 128],
                        Wb_s[:, 2048 + 512 * g:2048 + 512 * g + 512],
                        start=False, stop=True)
                nc.vector.tensor_copy(
                    v_all[:, 1024 * s4 + 512 * g:1024 * s4 + 512 * g + 512], pv[:])
        if not bench_iters:
            wpool.release()
        work = stack.enter_context(tc.tile_pool(name="work", bufs=2))
        expT_bufs = 1 if bench_iters else 2

        # ---------- B. self term for ALL slots ----------
        # selfW[rows, h] = sum_d q*k; T0/T8 row-tiles get separate banks.
        sq_all = res.tile([128, 4096], BF, tag="sq_all")
        nc.vector.tensor_mul(sq_all[:], qkT_all[:, 0:4096], qkT_all[:, 4096:8192])
        sqv = sq_all[:].rearrange("p (t r) -> p t r", t=8)
        expSelfA = res.tile([128, 64], BF, tag="expSelfA")
        for s4 in range(4):
            selfE = ps.tile([128, 512], F32, tag="ps")
            selfO = ps.tile([128, 512], F32, tag="ps")
            for p in range(8):
                for half, tgt in ((0, selfE), (1, selfO)):
                    nc.tensor.matmul(
                        tgt[:, p:p + 1],
                        sqv[64 * half:64 * half + 64, p,
                            128 * s4:128 * s4 + 128],
                        ones_col[64 * half:64 * half + 64, :],
                        start=True, stop=True)
            eSv = expSelfA[:, 16 * s4:16 * s4 + 16].rearrange(
                "p (a j) -> p a j", j=2)
            nc.scalar.activation(eSv[:, :, 0], selfE[:, 0:8], AF.Exp, scale=SCALE)
            nc.scalar.activation(eSv[:, :, 1], selfO[:, 0:8], AF.Exp, scale=SCALE)

        # ---------- C. prompt scores for ALL slots ----------
        expPA = res.tile([64, 4 * 2048], BF, tag="expPA")
        for s4 in range(4):
            qs4 = qkT_all[:].rearrange("p (t r) -> p t r", t=16)[
                :, :, 128 * s4:128 * s4 + 128]
            for half in range(2):
                for grp in range(2):
                    pp = ps.tile([128, 512], F32, tag="ps")
                    for i in range(4):
                        pr = 4 * grp + i
                        nc.tensor.matmul(
                            pp[0:64, 128 * i:128 * i + 128],
                            pKT_s[64 * half:64 * half + 64, 64 * pr:64 * pr + 64],
                            qs4[64 * half:64 * half + 64, pr, :],
                            start=True, stop=True)
                    if has_pmask:
                        nc.vector.tensor_add(
                            pp[0:64, 0:512].rearrange("p (i r) -> p i r", i=4),
                            pp[0:64, 0:512].rearrange("p (i r) -> p i r", i=4),
                            pM_s[:, 128 * s4:128 * s4 + 128].rearrange(
                                "p (i r) -> p i r", i=1).broadcast_to((64, 4, 128)))
                    nc.scalar.activation(
                        expPA[:, 2048 * s4:2048 * s4 + 2048].rearrange(
                            "p (pr c) -> p pr c", pr=8)[
                            :, 4 * grp:4 * grp + 4,
                            128 * half:128 * half + 128],
                        pp[0:64, 0:512].rearrange("p (i c) -> p i c", i=4),
                        AF.Exp, scale=SCALE)

        for s in range(4):
            ext = EXT[s]
            # per-slot views into the hoisted projections
            qs = qkT_all[:].rearrange("p (t r) -> p t r", t=16)[:, :, 128 * s:128 * s + 128]
            v2 = v_all[:, 1024 * s:1024 * s + 1024]
            expSelfN = expSelfA[:, 16 * s:16 * s + 16]
            expP = expPA[:, 2048 * s:2048 * s + 2048]

            # ---------- D. attention pair loop ----------
            attnF = work.tile([128, 1024], F32, tag="attnF", bufs=expT_bufs)
            den2 = work.tile([128, 16], F32, tag="den2")
            expT = work.tile([128, 16 * 8 * 128], BF, tag="expT", bufs=expT_bufs)
            t1 = work.tile([128, 1024], F32, tag="t1", bufs=expT_bufs)
            t1v = t1[:].rearrange("p (i c) -> p i c", i=16)
            nc.vector.tensor_tensor(
                t1v, v2.rearrange("p (i c) -> p i c", i=16),
                expSelfN[:].rearrange("p (i o) -> p i o", o=1).broadcast_to(
                    (128, 16, 64)),
                op=mybir.AluOpType.mult)

            for p in range(8):
                h0, h1 = 2 * p, 2 * p + 1
                # --- QK text: per head, banks of up to 4 key-blocks ---
                for g in range((ext + 3) // 4):
                    k0 = 4 * g
                    nkb = min(4, ext - k0)
                    sc0 = ps.tile([128, 512], F32, tag="ps")
                    sc1 = ps.tile([128, 512], F32, tag="ps")
                    for half, sc in ((0, sc0), (1, sc1)):
                        for i in range(nkb):
                            kb = k0 + i
                            nc.tensor.matmul(
                                sc[:, 128 * i:128 * i + 128],
                                KT_s[64 * half:64 * half + 64,
                                     1024 * p + 128 * kb:1024 * p + 128 * kb + 128],
                                qs[64 * half:64 * half + 64, p, :],
                                start=True, stop=True)
                    for h, sc in ((h0, sc0), (h1, sc1)):
                        nc.scalar.activation(
                            expT[:, (h * 8 + k0) * 128:(h * 8 + k0 + nkb) * 128],
                            sc[:, 0:128 * nkb], AF.Exp, scale=SCALE)
                # --- multiplicative causal/pad mask on last two key-blocks ---
                m = mQ_s[:, 256 * s:256 * s + 256]
                for h in (h0, h1):
                    e = expT[:, (h * 8 + ext - 2) * 128:(h * 8 + ext) * 128]
                    nc.vector.tensor_mul(e, e, m)
                # --- AV accumulate [rows, 65] per head ---
                av = ps.tile([128, 512], F32, tag="ps")
                for i, h in ((0, h0), (1, h1)):
                    o = av[:, 256 * i:256 * i + 65]
                    for kb in range(ext):
                        nc.tensor.matmul(
                            o, expT[:, (h * 8 + kb) * 128:(h * 8 + kb) * 128 + 128],
                            Vp_s[:, (h * 8 + kb) * 65:(h * 8 + kb) * 65 + 65],
                            start=(kb == 0), stop=False)
                    nc.tensor.matmul(
                        o, expP[:, 128 * h:128 * h + 128],
                        pVp_s[:, 65 * h:65 * h + 65],
                        start=False, stop=True)
                # --- epilogue: add self contribution ---
                avv = av[:].rearrange("p (i c) -> p i c", i=2)
                nc.vector.tensor_tensor(
                    attnF[:, 128 * p:128 * p + 128].rearrange(
                        "p (i c) -> p i c", i=2),
                    avv[:, :, 0:64],
                    t1[:, 128 * p:128 * p + 128].rearrange("p (i c) -> p i c", i=2),
                    op=mybir.AluOpType.add)
                nc.vector.tensor_tensor(
                    den2[:, 2 * p:2 * p + 2].rearrange("p (i o) -> p i o", o=1),
                    avv[:, :, 64:65],
                    expSelfN[:, 2 * p:2 * p + 2].rearrange("p (i o) -> p i o", o=1),
                    op=mybir.AluOpType.add)

            # ---------- E. divide + transpose + c_proj ----------
            rec = work.tile([128, 16], F32, tag="rec")
            nc.vector.reciprocal(rec[:], den2[:])
            attnO = work.tile([128, 1024], BF, tag="attnO", bufs=expT_bufs)
            nc.vector.tensor_tensor(
                attnO[:].rearrange("p (h c) -> p h c", h=16),
                attnF[:].rearrange("p (h c) -> p h c", h=16),
                rec[:].rearrange("p (h o) -> p h o", o=1).broadcast_to((128, 16, 64)),
                op=mybir.AluOpType.mult)
            attnT = work.tile([128, 8 * 128], BF, tag="attnT", bufs=expT_bufs)
            for gt in range(2):
                pt = ps.tile([128, 1024], BF, tag="ps")
                for e in range(4):
                    nc.tensor.transpose(
                        pt[:, 128 * e:128 * e + 128],
                        attnO[:, 128 * (4 * gt + e):128 * (4 * gt + e) + 128],
                        ident[:])
                nc.vector.tensor_copy(
                    attnT[:, 512 * gt:512 * gt + 512], pt[:, 0:512])
            outS = work.tile([128, 1024], F32, tag="outS", bufs=expT_bufs)
            for g in range(2):
                po = ps.tile([128, 512], F32, tag="ps")
                for e in range(8):
                    nc.tensor.matmul(
                        po[:], attnT[:, 128 * e:128 * e + 128],
                        Wp_s[:, e * 1024 + 512 * g: e * 1024 + 512 * g + 512],
                        start=(e == 0), stop=(e == 7 and not has_bias))
                if has_bias:
                    nc.tensor.matmul(
                        po[:], ones_row[0:1, 0:128],
                        Wpb_s[:, 512 * g:512 * g + 512], start=False, stop=True)
                nc.vector.tensor_copy(outS[:, 512 * g:512 * g + 512], po[:])
            nc.sync.dma_start(d_out[s], outS[:])

    nc.finalize()
    return nc


def prep_inputs(hidden_states, promptKey, promptValue, textualKey, textualValue,
                promptMask, c_attn_w, c_attn_b, c_proj_w, c_proj_b):
    """Build per-core input dicts (host-side shard + transpose + bf16 cast).

    Returns (in_maps, has_bias, has_pmask)."""
    hs = np.asarray(hidden_states, np.float32)
    pK = np.asarray(promptKey, np.float32)
    pV = np.asarray(promptValue, np.float32)
    tK = np.asarray(textualKey, np.float32)
    tV = np.asarray(textualValue, np.float32)
    pM = np.asarray(promptMask, bool)
    W = np.asarray(c_attn_w, np.float32)
    Wb = np.asarray(c_attn_b, np.float32)
    Wp = np.asarray(c_proj_w, np.float32)
    Wpb = np.asarray(c_proj_b, np.float32)

    has_bias = bool(np.any(Wb) or np.any(Wpb))
    has_pmask = not bool(pM.all())

    W8 = np.ascontiguousarray(
        W.reshape(8, 128, 3072).transpose(1, 0, 2)).astype(bf16)
    Wp8 = np.ascontiguousarray(
        Wp.reshape(8, 128, 1024).transpose(1, 0, 2)).astype(bf16)

    in_maps = []
    for c in range(NC):
        b, half = c // 2, c % 2
        qb = QBLOCKS[half]
        rows = np.concatenate([np.arange(128 * q, 128 * q + 128) for q in qb])
        xT = np.ascontiguousarray(
            hs[b][rows].T.reshape(8, 128, 512).transpose(1, 0, 2)).astype(bf16)
        KT = np.ascontiguousarray(
            tK[b].transpose(0, 2, 1).reshape(8, 128, 1024).transpose(1, 0, 2)
        ).astype(bf16)
        Vp = np.ones((128, 16, 8, 65), np.float32)
        Vp[:, :, :, 0:64] = tV[b].reshape(16, 8, 128, 64).transpose(2, 0, 1, 3)
        Vp = Vp.astype(bf16)
        pKT = np.ascontiguousarray(
            pK[b].transpose(0, 2, 1).reshape(8, 128, 64).transpose(1, 0, 2)
        ).astype(bf16)
        pVp = np.ones((64, 16, 65), np.float32)
        pVp[:, :, 0:64] = pV[b].transpose(1, 0, 2)
        pVp = pVp.astype(bf16)
        # multiplicative {0,1} mask for the last two key-blocks of each slot
        mQ = np.empty((128, 4, 256), np.float32)
        for s in range(4):
            Q = qb[s]
            for j in range(2):
                kb = EXT[s] - 2 + j
                keyabs = 128 * kb + np.arange(128)[:, None]
                rowabs = 128 * Q + np.arange(128)[None, :]
                mQ[:, s, 128 * j:128 * j + 128] = (keyabs < rowabs)
        im = {
            "xT": xT, "W": W8, "Wp": Wp8,
            "KT": KT, "Vp": Vp, "pKT": pKT, "pVp": pVp,
            "mQ": mQ.astype(bf16),
        }
        if has_bias:
            im["Wb"] = Wb.reshape(1, 3072).astype(bf16)
            im["Wpb"] = Wpb.reshape(1, 1024).astype(bf16)
        if has_pmask:
            pMb = np.empty((64, 4, 128), np.float32)
            for s in range(4):
                Q = qb[s]
                pMb[:, s, :] = np.where(
                    pM[b, 0, 128 * Q:128 * Q + 128, :].T, 0.0, -10000.0)
            im["pM"] = pMb
        in_maps.append(im)
    return in_maps, has_bias, has_pmask


def unshard(results):
    out = np.empty((B, S, E), np.float32)
    for c in range(NC):
        b, half = c // 2, c % 2
        o = results[c]["out"]
        for s in range(4):
            Q = QBLOCKS[half][s]
            out[b, 128 * Q:128 * Q + 128, :] = o[s]
    return out


_nc_cache = {}


def kernel(**inputs):
    in_maps, has_bias, has_pmask = prep_inputs(**inputs)
    key = (has_bias, has_pmask)
    if key not in _nc_cache:
        _nc_cache[key] = build_program(*key)
    res = run_bass_kernel_spmd(_nc_cache[key], in_maps, list(range(NC)))
    return unshard(res.results)

